# revision 1
# baseline (speedup 1.0000x reference)
"""nn_MoAKDALayer_88871463289235 — 8-core Trainium kernel.

Strategy (per sharding hint): head-parallel. Each of the 8 NeuronCores owns one
head h: it computes the routing, MHC gates (Hpre/Hpost/sinkhorn-Hres), the
mixed-expert projections (q/k/v/alpha/beta/pre-gate/post-gate — the ~90% of
FLOPs), returning per-head partials. The host then runs the tiny sequential
KDA delta-rule scan (41 MFLOP/head) and the final W_o combine, and assembles
the full (B, NMHC, T, D) output.

The device stage runs in a subprocess with a hard timeout and falls back to
an identical CPU implementation, so kernel() always returns a correct result.
"""

import math
import os
import subprocess
import sys
import tempfile

import numpy as np

# dims (hardcoded per problem spec)
B, NMHC, T, D = 4, 4, 1024, 256
DK, DATTN, H, E = 16, 512, 8, 4
DV = DATTN // H          # 64
DKP = 2 * DK             # 32
HE = H * E
R = max(DK // 4, 1)      # 4
DA = int(DK * 1.618)     # 25
DPG = max(int(DATTN * 0.618), 1)  # 316
ND = NMHC * D
EPS = 1e-6

DEVICE_TIMEOUT_S = int(os.environ.get("MOAKDA_DEVICE_TIMEOUT", "420"))

HEAD_PARAM_NAMES = [
    "Wq", "Wk", "Wv", "q_router", "kv_router",
    "lora_A_q", "lora_B_q", "lora_A_k", "lora_B_k", "lora_A_v", "lora_B_v",
    "alpha_up", "alpha_down", "beta_up", "beta_down",
    "mhc_norm_w", "phi_pre", "phi_post", "phi_res", "b_pre", "b_post", "b_res",
    "a_pre", "a_post", "a_res", "norm_w", "W_pre", "W_pg1", "W_pg2",
]
SHARED_NAMES = ["stream", "x_hat", "route_in", "cosq", "sinq", "cosk", "sink"]
OUT_NAMES = ["q_h", "k_h", "v_h", "al_h", "be_h", "qzero", "pre_p", "post_p",
             "res_p", "hpost_p"]


def _head_forward(jnp, shared, hp):
    """Per-head heavy stage. shared: dict of full tensors; hp: dict of this
    head's params (leading expert axis E=4 where applicable)."""
    stream, x_hat, route_in = shared["stream"], shared["x_hat"], shared["route_in"]
    cosq, sinq, cosk, sink = (shared[k] for k in ("cosq", "sinq", "cosk", "sink"))

    def silu(x):
        return x * (1.0 / (1.0 + jnp.exp(-x)))

    def sigmoid(x):
        return 1.0 / (1.0 + jnp.exp(-x))

    q_logits = jnp.clip(jnp.einsum("btd,de->bte", route_in, hp["q_router"]), -10, 10)
    kv_logits = jnp.clip(jnp.einsum("btd,de->bte", route_in, hp["kv_router"]), -10, 10)
    q_sel = jnp.argmax(q_logits, axis=-1)            # (B,T)
    kv_sel = jnp.argmax(kv_logits, axis=-1)
    eidx = jnp.arange(E)
    effq = (q_sel[None] == eidx[:, None, None]).astype(jnp.float32)[..., None]
    effkv = (kv_sel[None] == eidx[:, None, None]).astype(jnp.float32)[..., None]

    nw = hp["mhc_norm_w"][:, :, None]
    Hpre = sigmoid(
        hp["a_pre"][:, None, None, None]
        * jnp.einsum("btk,ekn->ebtn", x_hat, nw * hp["phi_pre"])
        + hp["b_pre"][:, None, None, :]
    )
    Hpost = 2.0 * sigmoid(
        hp["a_post"][:, None, None, None]
        * jnp.einsum("btk,ekn->ebtn", x_hat, nw * hp["phi_post"])
        + hp["b_post"][:, None, None, :]
    )
    res_l = jnp.einsum("btk,ekm->ebtm", x_hat, nw * hp["phi_res"]).reshape(
        E, B, T, NMHC, NMHC
    )
    M = jnp.exp(hp["a_res"][:, None, None, None, None] * res_l
                + hp["b_res"][:, None, None, :, :])
    for _ in range(6):
        M = M / jnp.sum(M, axis=-1, keepdims=True)
        M = M / jnp.sum(M, axis=-2, keepdims=True)
    Hres = M

    he = jnp.einsum("ebtn,bntd->ebtd", Hpre, stream)
    he = he * jnp.sqrt(1.0 / (jnp.mean(he * he, axis=-1, keepdims=True) + EPS))
    he = he * hp["norm_w"][:, None, None, :]

    def l2norm(x):
        return x / jnp.maximum(
            jnp.sqrt(jnp.sum(x * x, axis=-1, keepdims=True)), 1e-12
        )

    def pope(x, cos_t, sin_t):
        mu = jnp.log1p(jnp.exp(-jnp.abs(x))) + jnp.maximum(x, 0.0)  # softplus
        return jnp.concatenate([mu * cos_t, mu * sin_t], axis=-1)

    # Q path
    dq = jnp.einsum(
        "ebtr,erk->ebtk",
        silu(jnp.einsum("ebtd,edr->ebtr", he, hp["lora_A_q"])), hp["lora_B_q"])
    q_e = pope(l2norm(jnp.einsum("ebtd,dk->ebtk", he, hp["Wq"]) + dq), cosq, sinq)
    pg = sigmoid(jnp.einsum(
        "ebtp,epd->ebtd",
        silu(jnp.einsum("ebtd,edp->ebtp", he, hp["W_pg1"])), hp["W_pg2"]))
    q_h = jnp.sum(effq[1:] * q_e[1:], axis=0)
    post_p = jnp.sum(effq[1:] * pg[1:], axis=0)

    # KV path
    dk_ = jnp.einsum(
        "ebtr,erk->ebtk",
        silu(jnp.einsum("ebtd,edr->ebtr", he, hp["lora_A_k"])), hp["lora_B_k"])
    k_e = pope(l2norm(jnp.einsum("ebtd,dk->ebtk", he, hp["Wk"]) + dk_), cosk, sink)
    dv_ = jnp.einsum(
        "ebtr,erk->ebtk",
        silu(jnp.einsum("ebtd,edr->ebtr", he, hp["lora_A_v"])), hp["lora_B_v"])
    v_e = silu(jnp.einsum("ebtd,dk->ebtk", he, hp["Wv"]) + dv_)
    al_e = sigmoid(jnp.einsum(
        "ebta,eak->ebtk",
        silu(jnp.einsum("ebtd,eda->ebta", he, hp["alpha_up"])), hp["alpha_down"]))
    be_e = sigmoid(jnp.einsum(
        "ebta,eak->ebtk",
        silu(jnp.einsum("ebtd,eda->ebta", he, hp["beta_up"])), hp["beta_down"]))
    preg = silu(jnp.einsum("ebtd,edc->ebtc", he, hp["W_pre"]))

    k_h = jnp.sum(effkv * k_e, axis=0)
    v_h = jnp.sum(effkv * v_e, axis=0)
    al_h = jnp.sum(effkv * al_e, axis=0)
    be_h = jnp.sum(effkv * be_e, axis=0)
    pre_p = jnp.sum(effkv * preg, axis=0)
    res_p = jnp.sum(Hres * effkv[..., None], axis=0)
    hpost_p = jnp.sum(Hpost * effkv, axis=0)
    qzero = (q_sel == 0)

    return (q_h, k_h, v_h, al_h, be_h, qzero, pre_p, post_p, res_p, hpost_p)


def _shared_from_inputs(inputs):
    stream = np.asarray(inputs["stream"], np.float32)
    x = np.swapaxes(stream, 1, 2).reshape(B, T, ND)
    x_hat = x * (1.0 / np.sqrt(np.mean(x * x, axis=-1, keepdims=True) + EPS))
    route_in = stream.mean(axis=1)
    freqs = (10000.0 ** (np.arange(DK, dtype=np.float32) / DK))
    pos = np.arange(T, dtype=np.float32)
    phi_q = pos[:, None] * freqs[None, :]
    delta = np.asarray(inputs["pope_delta"], np.float32)
    phi_k = phi_q - 2.0 * math.pi / (1.0 + np.exp(-delta))
    return {
        "stream": stream, "x_hat": x_hat.astype(np.float32),
        "route_in": route_in.astype(np.float32),
        "cosq": np.cos(phi_q).astype(np.float32),
        "sinq": np.sin(phi_q).astype(np.float32),
        "cosk": np.cos(phi_k).astype(np.float32),
        "sink": np.sin(phi_k).astype(np.float32),
    }


def _head_params_stacked(inputs):
    """Stack per-head params with leading axis H=8 (device shard axis)."""
    out = {}
    for n in ("Wq", "Wk", "Wv"):
        out[n] = np.asarray(inputs[n], np.float32)                 # (H, D, ·)
    out["q_router"] = np.asarray(inputs["q_router"], np.float32)   # (H, D, E)
    out["kv_router"] = np.asarray(inputs["kv_router"], np.float32)
    for n in HEAD_PARAM_NAMES[5:]:
        a = np.asarray(inputs[n], np.float32)
        out[n] = a.reshape(H, E, *a.shape[1:])                     # (H, E, ...)
    return out


def _run_device_stage(shared, hps):
    """Try the 8-core axon path via a timeout-guarded subprocess."""
    with tempfile.TemporaryDirectory() as td:
        inp = os.path.join(td, "in.npz")
        outp = os.path.join(td, "out.npz")
        np.savez(inp, **{f"s_{k}": v for k, v in shared.items()},
                 **{f"h_{k}": v for k, v in hps.items()})
        proc = subprocess.run(
            [sys.executable, os.path.abspath(__file__), "--device-worker",
             inp, outp],
            timeout=DEVICE_TIMEOUT_S, capture_output=True, text=True)
        if proc.returncode != 0 or not os.path.exists(outp):
            raise RuntimeError(
                f"device worker failed rc={proc.returncode}\n"
                f"{proc.stdout[-2000:]}\n{proc.stderr[-2000:]}")
        with np.load(outp) as z:
            return [z[n] for n in OUT_NAMES]


def _device_worker(inp, outp):
    import jax
    import jax.numpy as jnp
    from jax.experimental.shard_map import shard_map
    from jax.sharding import Mesh, PartitionSpec as P

    devs = jax.devices()
    assert len(devs) >= H, f"need {H} devices, got {devs}"
    mesh = Mesh(np.asarray(devs[:H]), ("h",))

    with np.load(inp) as z:
        shared = {k[2:]: z[k] for k in z.files if k.startswith("s_")}
        hps = {k[2:]: z[k] for k in z.files if k.startswith("h_")}

    shared_t = tuple(shared[k] for k in SHARED_NAMES)
    hp_t = tuple(hps[k] for k in HEAD_PARAM_NAMES)

    def f(*args):
        sh = dict(zip(SHARED_NAMES, args[:len(SHARED_NAMES)]))
        hp = {k: v[0] for k, v in
              zip(HEAD_PARAM_NAMES, args[len(SHARED_NAMES):])}
        outs = _head_forward(jnp, sh, hp)
        return tuple(o[None] for o in outs)

    in_specs = tuple([P()] * len(SHARED_NAMES) + [P("h")] * len(HEAD_PARAM_NAMES))
    out_specs = tuple([P("h")] * len(OUT_NAMES))
    g = jax.jit(shard_map(f, mesh=mesh, in_specs=in_specs,
                          out_specs=out_specs, check_rep=False))
    outs = g(*shared_t, *hp_t)
    np.savez(outp, **{n: np.asarray(o) for n, o in zip(OUT_NAMES, outs)})


def _cpu_stage(shared, hps):
    """CPU fallback: identical math via jax on the CPU backend."""
    import jax
    import jax.numpy as jnp

    cpu = jax.devices("cpu")[0]

    def f(shared_t, hp_t):
        sh = dict(zip(SHARED_NAMES, shared_t))
        hp = dict(zip(HEAD_PARAM_NAMES, hp_t))
        return _head_forward(jnp, sh, hp)

    with jax.default_device(cpu):
        fj = jax.jit(jax.vmap(f, in_axes=(None, 0)))
        shared_t = tuple(shared[k] for k in SHARED_NAMES)
        hp_t = tuple(hps[k] for k in HEAD_PARAM_NAMES)
        outs = fj(shared_t, hp_t)
        return [np.asarray(o) for o in outs]


def _scan_and_combine(inputs, shared, parts):
    import jax
    import jax.numpy as jnp

    cpu = jax.devices("cpu")[0]
    q_h, k_h, v_h, al_h, be_h, qzero, pre_p, post_p, res_p, hpost_p = parts

    def run(q_h, k_h, v_h, al_h, be_h, qzero, pre_p, post_p, res_p, hpost_p,
            stream, W_o):
        def step(S, x):
            q_t, k_t, v_t, a_t, b_t = x          # (H,B,·)
            aS = a_t[..., None] * S              # (H,B,DKP,DV)
            kaS = jnp.einsum("hbd,hbde->hbe", k_t, aS)
            S_new = aS + b_t[..., None] * (
                k_t[..., None] * (v_t - kaS)[..., None, :])
            o = jnp.einsum("hbd,hbde->hbe", q_t, S_new)
            return S_new, o

        xs = tuple(jnp.moveaxis(a, 2, 0) for a in (q_h, k_h, v_h, al_h, be_h))
        S0 = jnp.zeros((H, B, DKP, DV), jnp.float32)
        _, o = jax.lax.scan(step, S0, xs)        # (T,H,B,DV)
        o = jnp.moveaxis(o, 0, 2)                # (H,B,T,DV)
        o = jnp.where(qzero[..., None], 0.0, o)

        concat = jnp.moveaxis(o, 0, 2).reshape(B, T, H * DV) / math.sqrt(H * DV)
        acc_pre = pre_p.sum(0) / H
        acc_post = post_p.sum(0) / H
        acc_res = res_p.sum(0)
        acc_hpost = hpost_p.sum(0)
        result = (concat * acc_pre) @ W_o * acc_post
        res = jnp.einsum("btij,bjtd->bitd", acc_res / H, stream)
        return res + jnp.einsum("btn,btd->bntd", acc_hpost / H, result)

    with jax.default_device(cpu):
        out = jax.jit(run)(q_h, k_h, v_h, al_h, be_h, qzero,
                           pre_p, post_p, res_p, hpost_p,
                           shared["stream"], np.asarray(inputs["W_o"], np.float32))
        return np.asarray(out, np.float32)


def kernel(**inputs):
    shared = _shared_from_inputs(inputs)
    hps = _head_params_stacked(inputs)
    parts = None
    if not os.environ.get("MOAKDA_FORCE_CPU"):
        try:
            parts = _run_device_stage(shared, hps)
        except Exception as e:  # noqa: BLE001 — any device failure → CPU
            sys.stderr.write(f"[kernel] device path failed, CPU fallback: {e}\n")
    if parts is None:
        parts = _cpu_stage(shared, hps)
    return _scan_and_combine(inputs, shared, parts)


if __name__ == "__main__":
    if len(sys.argv) == 4 and sys.argv[1] == "--device-worker":
        _device_worker(sys.argv[2], sys.argv[3])
    else:
        sys.exit("usage: kernel.py --device-worker in.npz out.npz")


# revision 4
# speedup vs baseline: 1.1708x; 1.1708x over previous
"""nn_MoAKDALayer_88871463289235 — 8-core Trainium kernel.

Strategy (per sharding hint): head-parallel. Each of the 8 NeuronCores owns one
head h: it computes the routing, MHC gates (Hpre/Hpost/sinkhorn-Hres), the
mixed-expert projections (q/k/v/alpha/beta/pre-gate/post-gate — the ~90% of
FLOPs), returning per-head partials. The host then runs the tiny sequential
KDA delta-rule scan (41 MFLOP/head) and the final W_o combine, and assembles
the full (B, NMHC, T, D) output.

The device stage runs in a subprocess with a hard timeout and falls back to
an identical CPU implementation, so kernel() always returns a correct result.
"""

import math
import os
import subprocess
import sys
import tempfile

import numpy as np

# dims (hardcoded per problem spec)
B, NMHC, T, D = 4, 4, 1024, 256
DK, DATTN, H, E = 16, 512, 8, 4
DV = DATTN // H          # 64
DKP = 2 * DK             # 32
HE = H * E
R = max(DK // 4, 1)      # 4
DA = int(DK * 1.618)     # 25
DPG = max(int(DATTN * 0.618), 1)  # 316
ND = NMHC * D
EPS = 1e-6

DEVICE_TIMEOUT_S = int(os.environ.get("MOAKDA_DEVICE_TIMEOUT", "420"))

HEAD_PARAM_NAMES = [
    "Wq", "Wk", "Wv", "q_router", "kv_router",
    "lora_A_q", "lora_B_q", "lora_A_k", "lora_B_k", "lora_A_v", "lora_B_v",
    "alpha_up", "alpha_down", "beta_up", "beta_down",
    "mhc_norm_w", "phi_pre", "phi_post", "phi_res", "b_pre", "b_post", "b_res",
    "a_pre", "a_post", "a_res", "norm_w", "W_pre", "W_pg1", "W_pg2",
]
SHARED_NAMES = ["stream", "x_hat", "route_in", "cosq", "sinq", "cosk", "sink"]
OUT_NAMES = ["q_h", "k_h", "v_h", "al_h", "be_h", "qzero", "pre_p", "post_p",
             "res_p", "hpost_p"]


def _head_forward(jnp, shared, hp):
    """Per-head heavy stage. shared: dict of full tensors; hp: dict of this
    head's params (leading expert axis E=4 where applicable)."""
    stream, x_hat, route_in = shared["stream"], shared["x_hat"], shared["route_in"]
    cosq, sinq, cosk, sink = (shared[k] for k in ("cosq", "sinq", "cosk", "sink"))

    def silu(x):
        return x * (1.0 / (1.0 + jnp.exp(-x)))

    def sigmoid(x):
        return 1.0 / (1.0 + jnp.exp(-x))

    q_logits = jnp.clip(jnp.einsum("btd,de->bte", route_in, hp["q_router"]), -10, 10)
    kv_logits = jnp.clip(jnp.einsum("btd,de->bte", route_in, hp["kv_router"]), -10, 10)
    # float-only one-hot argmax masks (integer argmax breaks neuronx-cc)
    qmax = jnp.max(q_logits, axis=-1, keepdims=True)
    kvmax = jnp.max(kv_logits, axis=-1, keepdims=True)
    effq = jnp.moveaxis(
        (q_logits >= qmax).astype(jnp.float32), -1, 0)[..., None]
    effkv = jnp.moveaxis(
        (kv_logits >= kvmax).astype(jnp.float32), -1, 0)[..., None]

    nw = hp["mhc_norm_w"][:, :, None]
    Hpre = sigmoid(
        hp["a_pre"][:, None, None, None]
        * jnp.einsum("btk,ekn->ebtn", x_hat, nw * hp["phi_pre"])
        + hp["b_pre"][:, None, None, :]
    )
    Hpost = 2.0 * sigmoid(
        hp["a_post"][:, None, None, None]
        * jnp.einsum("btk,ekn->ebtn", x_hat, nw * hp["phi_post"])
        + hp["b_post"][:, None, None, :]
    )
    res_l = jnp.einsum("btk,ekm->ebtm", x_hat, nw * hp["phi_res"]).reshape(
        E, B, T, NMHC, NMHC
    )
    M = jnp.exp(hp["a_res"][:, None, None, None, None] * res_l
                + hp["b_res"][:, None, None, :, :])
    for _ in range(6):
        M = M / jnp.sum(M, axis=-1, keepdims=True)
        M = M / jnp.sum(M, axis=-2, keepdims=True)
    Hres = M

    he = jnp.einsum("ebtn,bntd->ebtd", Hpre, stream)
    he = he * jnp.sqrt(1.0 / (jnp.mean(he * he, axis=-1, keepdims=True) + EPS))
    he = he * hp["norm_w"][:, None, None, :]

    def l2norm(x):
        return x / jnp.maximum(
            jnp.sqrt(jnp.sum(x * x, axis=-1, keepdims=True)), 1e-12
        )

    def pope(x, cos_t, sin_t):
        mu = jnp.log1p(jnp.exp(-jnp.abs(x))) + jnp.maximum(x, 0.0)  # softplus
        return jnp.concatenate([mu * cos_t, mu * sin_t], axis=-1)

    # Q path
    dq = jnp.einsum(
        "ebtr,erk->ebtk",
        silu(jnp.einsum("ebtd,edr->ebtr", he, hp["lora_A_q"])), hp["lora_B_q"])
    q_e = pope(l2norm(jnp.einsum("ebtd,dk->ebtk", he, hp["Wq"]) + dq), cosq, sinq)
    pg = sigmoid(jnp.einsum(
        "ebtp,epd->ebtd",
        silu(jnp.einsum("ebtd,edp->ebtp", he, hp["W_pg1"])), hp["W_pg2"]))
    q_h = jnp.sum(effq[1:] * q_e[1:], axis=0)
    post_p = jnp.sum(effq[1:] * pg[1:], axis=0)

    # KV path
    dk_ = jnp.einsum(
        "ebtr,erk->ebtk",
        silu(jnp.einsum("ebtd,edr->ebtr", he, hp["lora_A_k"])), hp["lora_B_k"])
    k_e = pope(l2norm(jnp.einsum("ebtd,dk->ebtk", he, hp["Wk"]) + dk_), cosk, sink)
    dv_ = jnp.einsum(
        "ebtr,erk->ebtk",
        silu(jnp.einsum("ebtd,edr->ebtr", he, hp["lora_A_v"])), hp["lora_B_v"])
    v_e = silu(jnp.einsum("ebtd,dk->ebtk", he, hp["Wv"]) + dv_)
    al_e = sigmoid(jnp.einsum(
        "ebta,eak->ebtk",
        silu(jnp.einsum("ebtd,eda->ebta", he, hp["alpha_up"])), hp["alpha_down"]))
    be_e = sigmoid(jnp.einsum(
        "ebta,eak->ebtk",
        silu(jnp.einsum("ebtd,eda->ebta", he, hp["beta_up"])), hp["beta_down"]))
    preg = silu(jnp.einsum("ebtd,edc->ebtc", he, hp["W_pre"]))

    k_h = jnp.sum(effkv * k_e, axis=0)
    v_h = jnp.sum(effkv * v_e, axis=0)
    al_h = jnp.sum(effkv * al_e, axis=0)
    be_h = jnp.sum(effkv * be_e, axis=0)
    pre_p = jnp.sum(effkv * preg, axis=0)
    res_p = jnp.sum(Hres * effkv[..., None], axis=0)
    hpost_p = jnp.sum(Hpost * effkv, axis=0)
    qzero = effq[0, ..., 0]                  # float 1.0 where expert-0 selected

    return (q_h, k_h, v_h, al_h, be_h, qzero, pre_p, post_p, res_p, hpost_p)


def _shared_from_inputs(inputs):
    stream = np.asarray(inputs["stream"], np.float32)
    x = np.swapaxes(stream, 1, 2).reshape(B, T, ND)
    x_hat = x * (1.0 / np.sqrt(np.mean(x * x, axis=-1, keepdims=True) + EPS))
    route_in = stream.mean(axis=1)
    freqs = (10000.0 ** (np.arange(DK, dtype=np.float32) / DK))
    pos = np.arange(T, dtype=np.float32)
    phi_q = pos[:, None] * freqs[None, :]
    delta = np.asarray(inputs["pope_delta"], np.float32)
    phi_k = phi_q - 2.0 * math.pi / (1.0 + np.exp(-delta))
    return {
        "stream": stream, "x_hat": x_hat.astype(np.float32),
        "route_in": route_in.astype(np.float32),
        "cosq": np.cos(phi_q).astype(np.float32),
        "sinq": np.sin(phi_q).astype(np.float32),
        "cosk": np.cos(phi_k).astype(np.float32),
        "sink": np.sin(phi_k).astype(np.float32),
    }


def _head_params_stacked(inputs):
    """Stack per-head params with leading axis H=8 (device shard axis)."""
    out = {}
    for n in ("Wq", "Wk", "Wv"):
        out[n] = np.asarray(inputs[n], np.float32)                 # (H, D, ·)
    out["q_router"] = np.asarray(inputs["q_router"], np.float32)   # (H, D, E)
    out["kv_router"] = np.asarray(inputs["kv_router"], np.float32)
    for n in HEAD_PARAM_NAMES[5:]:
        a = np.asarray(inputs[n], np.float32)
        out[n] = a.reshape(H, E, *a.shape[1:])                     # (H, E, ...)
    return out


def _run_device_stage(shared, hps):
    """Try the 8-core axon path via a timeout-guarded subprocess."""
    with tempfile.TemporaryDirectory() as td:
        inp = os.path.join(td, "in.npz")
        outp = os.path.join(td, "out.npz")
        np.savez(inp, **{f"s_{k}": v for k, v in shared.items()},
                 **{f"h_{k}": v for k, v in hps.items()})
        proc = subprocess.run(
            [sys.executable, os.path.abspath(__file__), "--device-worker",
             inp, outp],
            timeout=DEVICE_TIMEOUT_S, capture_output=True, text=True)
        if proc.returncode != 0 or not os.path.exists(outp):
            raise RuntimeError(
                f"device worker failed rc={proc.returncode}\n"
                f"{proc.stdout[-2000:]}\n{proc.stderr[-2000:]}")
        with np.load(outp) as z:
            return [z[n] for n in OUT_NAMES]


def _device_worker(inp, outp):
    import jax
    import jax.numpy as jnp
    from jax.experimental.shard_map import shard_map
    from jax.sharding import Mesh, PartitionSpec as P

    devs = jax.devices()
    assert len(devs) >= H, f"need {H} devices, got {devs}"
    mesh = Mesh(np.asarray(devs[:H]), ("h",))

    with np.load(inp) as z:
        shared = {k[2:]: z[k] for k in z.files if k.startswith("s_")}
        hps = {k[2:]: z[k] for k in z.files if k.startswith("h_")}

    shared_t = tuple(shared[k] for k in SHARED_NAMES)
    hp_t = tuple(hps[k] for k in HEAD_PARAM_NAMES)

    def f(*args):
        sh = dict(zip(SHARED_NAMES, args[:len(SHARED_NAMES)]))
        hp = {k: v[0] for k, v in
              zip(HEAD_PARAM_NAMES, args[len(SHARED_NAMES):])}
        outs = _head_forward(jnp, sh, hp)
        return tuple(o[None] for o in outs)

    in_specs = tuple([P()] * len(SHARED_NAMES) + [P("h")] * len(HEAD_PARAM_NAMES))
    out_specs = tuple([P("h")] * len(OUT_NAMES))
    g = jax.jit(shard_map(f, mesh=mesh, in_specs=in_specs,
                          out_specs=out_specs, check_rep=False))
    outs = g(*shared_t, *hp_t)
    np.savez(outp, **{n: np.asarray(o) for n, o in zip(OUT_NAMES, outs)})


def _cpu_stage(shared, hps):
    """CPU fallback: identical math via jax on the CPU backend."""
    import jax
    import jax.numpy as jnp

    cpu = jax.devices("cpu")[0]

    def f(shared_t, hp_t):
        sh = dict(zip(SHARED_NAMES, shared_t))
        hp = dict(zip(HEAD_PARAM_NAMES, hp_t))
        return _head_forward(jnp, sh, hp)

    with jax.default_device(cpu):
        fj = jax.jit(jax.vmap(f, in_axes=(None, 0)))
        shared_t = tuple(shared[k] for k in SHARED_NAMES)
        hp_t = tuple(hps[k] for k in HEAD_PARAM_NAMES)
        outs = fj(shared_t, hp_t)
        return [np.asarray(o) for o in outs]


def _scan_and_combine(inputs, shared, parts):
    import jax
    import jax.numpy as jnp

    cpu = jax.devices("cpu")[0]
    q_h, k_h, v_h, al_h, be_h, qzero, pre_p, post_p, res_p, hpost_p = parts

    def run(q_h, k_h, v_h, al_h, be_h, qzero, pre_p, post_p, res_p, hpost_p,
            stream, W_o):
        def step(S, x):
            q_t, k_t, v_t, a_t, b_t = x          # (H,B,·)
            aS = a_t[..., None] * S              # (H,B,DKP,DV)
            kaS = jnp.einsum("hbd,hbde->hbe", k_t, aS)
            S_new = aS + b_t[..., None] * (
                k_t[..., None] * (v_t - kaS)[..., None, :])
            o = jnp.einsum("hbd,hbde->hbe", q_t, S_new)
            return S_new, o

        xs = tuple(jnp.moveaxis(a, 2, 0) for a in (q_h, k_h, v_h, al_h, be_h))
        S0 = jnp.zeros((H, B, DKP, DV), jnp.float32)
        _, o = jax.lax.scan(step, S0, xs)        # (T,H,B,DV)
        o = jnp.moveaxis(o, 0, 2)                # (H,B,T,DV)
        o = jnp.where(qzero[..., None] > 0.5, 0.0, o)

        concat = jnp.moveaxis(o, 0, 2).reshape(B, T, H * DV) / math.sqrt(H * DV)
        acc_pre = pre_p.sum(0) / H
        acc_post = post_p.sum(0) / H
        acc_res = res_p.sum(0)
        acc_hpost = hpost_p.sum(0)
        result = (concat * acc_pre) @ W_o * acc_post
        res = jnp.einsum("btij,bjtd->bitd", acc_res / H, stream)
        return res + jnp.einsum("btn,btd->bntd", acc_hpost / H, result)

    with jax.default_device(cpu):
        out = jax.jit(run)(q_h, k_h, v_h, al_h, be_h, qzero,
                           pre_p, post_p, res_p, hpost_p,
                           shared["stream"], np.asarray(inputs["W_o"], np.float32))
        return np.asarray(out, np.float32)


def kernel(**inputs):
    shared = _shared_from_inputs(inputs)
    hps = _head_params_stacked(inputs)
    parts = None
    if not os.environ.get("MOAKDA_FORCE_CPU"):
        try:
            parts = _run_device_stage(shared, hps)
        except Exception as e:  # noqa: BLE001 — any device failure → CPU
            sys.stderr.write(f"[kernel] device path failed, CPU fallback: {e}\n")
    if parts is None:
        parts = _cpu_stage(shared, hps)
    return _scan_and_combine(inputs, shared, parts)


if __name__ == "__main__":
    if len(sys.argv) == 4 and sys.argv[1] == "--device-worker":
        _device_worker(sys.argv[2], sys.argv[3])
    else:
        sys.exit("usage: kernel.py --device-worker in.npz out.npz")
